# revision 9
# baseline (speedup 1.0000x reference)
"""Trainium2 Bass kernel for CovarianceSimilarity.

scores[n,w] = sum_k conv_w[k]*leaky_relu(diag(Qn^T cov_w Qn)[k]) + conv_b

Strategy (8 NeuronCores, data-parallel over NQ).  On this stack wall time
is dominated by per-instruction dispatch and per-byte emulation cost, so the
kernel minimizes instruction count (esp. matmuls/DMAs) and data movement:
  - Host prep (layout only + dtype cast): reshape/pad/shard query; transpose
    support chunks.  Queries ship as bf16 (halves DMA bytes); support ships
    as f32r (same bits as f32) so PE consumes it with no on-device
    conversion.  All matmuls run f32r: bf16 matmuls emit a separate
    InstLdweights per matmul on this backend (~2x instruction count, net
    slower despite cheaper streaming -- measured).
  - Cov phase: each core computes partial uncentered moments A_w = S S^T over
    its 1/8 chunk of the Shot*d sample axis (contraction along partitions --
    support fed pre-transposed, one 1.6MB DMA per way), plus sample row-sums
    via ones-matmuls.  One f32 AllReduce pair combines both.  Rank-1
    correction (mean removal) + triangular masking produce V tiles
    (lower-triangular, half diagonal, scaled by 2/(N-1+eps), f32r) so that
    diag(Qn^T cov Qn) = colsum(Qn o (V^T Qn)) with half the matmul FLOPs.
    V uses TRUE triangle column counts (no padded columns = no wasted MACs).
  - Main loop per (query q, k-block, way): Y'[k,c] = sum_b Qn[b,k] V[b,c]
    accumulated in PSUM over the 5 b-blocks (triangle-clipped free runs),
    then one fused tensor_tensor_reduce per k-block computes
    F[k] = sum_c Y'[k,c]*QnT[k,c].  LeakyReLU on the scalar engine, conv
    dot via another fused reduce, final partition-sum via a ones-matmul.
  - One big DMA per query for each of Qn-source and QnT-source (bf16,
    host-interleaved so each is a single contiguous [128, 5120] slab):
    46 DMA instructions total vs 222 in the padded-f32 version.
  - gpsimd engine unused (its tensor ops cost ~300us each here): QnT scaling
    runs on DVE; the inv-norm row broadcast is a PE ones-outer-product.
"""

import numpy as np
from contextlib import ExitStack

import concourse.bass as bass
import concourse.tile as tile
from concourse import bacc, mybir
from concourse.bass_utils import run_bass_kernel_spmd

# ---- problem constants (hardcoded per spec) ----
NQ, C, D = 75, 640, 1024
WAY, SHOT = 5, 5
NTOT = SHOT * D            # 5120 samples per way
NCORES = 8
QPC = 10                   # queries per core (NQ padded to 80)
NCHUNK = NTOT // NCORES    # 640 sample columns per core
EPS = 1e-8
NEG_SLOPE = 0.2
CB = C // 128              # 5 c-blocks
KB = D // 128              # 8 k-blocks
NB = NCHUNK // 128         # 5 n'-blocks per core
SCALE2 = 2.0 / (NTOT - 1 + EPS)
F32 = mybir.dt.float32
F32R = mybir.dt.float32r
BF16 = mybir.dt.bfloat16

# V tile column counts per block-row p: true triangle, cols [0,(p+1)*128)
VCOLS = [128, 256, 384, 512, 640]
# matmul free-dim runs per p (PSUM-bank-contained: 512 f32 per bank)
VRUNS = {0: [(0, 128)], 1: [(0, 256)], 2: [(0, 384)], 3: [(0, 512)],
         4: [(0, 512), (512, 640)]}
ACOLS = WAY * sum(VCOLS)   # 9600: AllReduce payload columns

_CACHE = {}
RG = [list(range(NCORES))]
SKIP_CC = False


def _build():
    nc = bacc.Bacc("TRN2", target_bir_lowering=False, debug=False,
                   num_devices=NCORES)
    qs_ap = nc.dram_tensor("qs", [QPC, 128, CB * D], BF16,
                           kind="ExternalInput").ap()
    qt_ap = nc.dram_tensor("qt", [QPC, 128, KB * C], BF16,
                           kind="ExternalInput").ap()
    st_ap = nc.dram_tensor("st", [128, WAY * NB * C], F32R,
                           kind="ExternalInput").ap()
    cw_ap = nc.dram_tensor("cw", [128, KB], F32, kind="ExternalInput").ap()
    cb_ap = nc.dram_tensor("cb", [1, 1], F32, kind="ExternalInput").ap()
    dm_ap = nc.dram_tensor("dm", [128, 128], F32, kind="ExternalInput").ap()
    id_ap = nc.dram_tensor("idm", [128, 128], F32, kind="ExternalInput").ap()
    out_ap = nc.dram_tensor("out", [1, QPC * WAY], F32,
                            kind="ExternalOutput").ap()

    with tile.TileContext(nc) as tc:
        with ExitStack() as ctx:
            _body(nc, tc, ctx, qs_ap, qt_ap, st_ap, cw_ap, cb_ap, dm_ap,
                  id_ap, out_ap)
    nc.compile()
    return nc


def _body(nc, tc, ctx, qs_ap, qt_ap, st_ap, cw_ap, cb_ap, dm_ap, id_ap,
          out_ap):
    # ---------------- persistent pools ----------------
    vpool = ctx.enter_context(tc.tile_pool(name="v", bufs=1))
    const = ctx.enter_context(tc.tile_pool(name="const", bufs=1))
    rpool = ctx.enter_context(tc.tile_pool(name="r", bufs=1))

    # constants
    ones32 = const.tile([128, 1], F32, tag="ones32")
    nc.vector.memset(ones32[:], 1.0)
    onesr = const.tile([128, 1], F32R, tag="onesr")
    nc.vector.tensor_copy(onesr[:], ones32[:])
    ones_row_f = const.tile([1, 128], F32, tag="onesrowf")
    nc.vector.memset(ones_row_f[:], 1.0)
    ones_row = const.tile([1, 128], BF16, tag="onesrow")
    nc.vector.tensor_copy(ones_row[:], ones_row_f[:])
    dmask = const.tile([128, 128], F32, tag="dmask")
    nc.sync.dma_start(dmask[:], dm_ap[:])
    idm = const.tile([128, 128], F32, tag="idm")
    nc.sync.dma_start(idm[:], id_ap[:])
    cw_sb = const.tile([128, KB], F32, tag="cw")
    nc.sync.dma_start(cw_sb[:], cw_ap[:])
    cb_sb = const.tile([1, 1], F32, tag="cb")
    nc.sync.dma_start(cb_sb[:], cb_ap[:])

    # V tiles (bf16), one per (way, block-row p)
    vt = [[vpool.tile([128, VCOLS[p]], F32R, tag=f"v{w}_{p}",
                      name=f"v{w}_{p}")
           for p in range(CB)] for w in range(WAY)]

    # conv partial accumulator: R[:, qw]
    r_acc = rpool.tile([128, QPC * WAY], F32, tag="racc")

    # ---------------- phase A: partial covariance ----------------
    with tc.tile_pool(name="covdram", bufs=1, space="DRAM") as cov_dram, \
         ExitStack() as cov_ctx:
        a_in = cov_dram.tile([128, ACOLS], F32)               # 4.9 MB
        a_out = cov_dram.tile([128, ACOLS], F32, addr_space="Shared")
        s_in = cov_dram.tile([1, WAY * C], F32)
        s_out = cov_dram.tile([1, WAY * C], F32, addr_space="Shared")

        stp = cov_ctx.enter_context(tc.tile_pool(name="straw", bufs=1))
        # one 1.6MB f32 DMA per way; slice nb = [:, nb*C:(nb+1)*C]
        st_w = []
        for w in range(WAY):
            t = stp.tile([128, NB * C], F32R, tag=f"st{w}")
            nc.sync.dma_start(t[:], st_ap[:, w * NB * C:(w + 1) * NB * C])
            st_w.append(t)

        with tc.tile_pool(name="apsum", bufs=2, space="PSUM") as apsum, \
             tc.tile_pool(name="spsum", bufs=2, space="PSUM") as spsum, \
             tc.tile_pool(name="sbdrain", bufs=2) as sbdrain:
            a_off = 0
            for w in range(WAY):
                def st_slc(nb, lo, hi):
                    return st_w[w][:, nb * C + lo:nb * C + hi]
                # s_w row: ones^T @ st  (accumulate over blocks)
                s_ps = spsum.tile([1, 640], F32, tag="sps")
                for nb in range(NB):
                    nc.tensor.matmul(s_ps[:, 0:512], onesr[:],
                                     st_slc(nb, 0, 512),
                                     start=(nb == 0), stop=(nb == NB - 1))
                    nc.tensor.matmul(s_ps[:, 512:640], onesr[:],
                                     st_slc(nb, 512, 640),
                                     start=(nb == 0), stop=(nb == NB - 1))
                s_row = sbdrain.tile([1, C], F32, tag="srow")
                nc.vector.tensor_copy(s_row[:], s_ps[:, 0:C])
                nc.sync.dma_start(s_in[0:1, w * C:(w + 1) * C], s_row[:])
                # A block-rows (true triangle), drain whole way at once
                a_sb = sbdrain.tile([128, sum(VCOLS)], F32, tag="adrain")
                d_off = 0
                for m in range(CB):
                    keep = (m + 1) * 128
                    runs = [(0, min(keep, 512))]
                    if keep > 512:
                        runs.append((512, keep))
                    a_ps = apsum.tile([128, 640], F32, tag="aps")
                    for nb in range(NB):
                        for lo, hi in runs:
                            nc.tensor.matmul(
                                a_ps[:, lo:hi],
                                st_slc(nb, m * 128, (m + 1) * 128),
                                st_slc(nb, lo, hi),
                                start=(nb == 0), stop=(nb == NB - 1))
                    nc.vector.tensor_copy(a_sb[:, d_off:d_off + keep],
                                          a_ps[:, 0:keep])
                    d_off += keep
                nc.sync.dma_start(a_in[:, a_off:a_off + d_off], a_sb[:])
                a_off += d_off

        if SKIP_CC:
            nc.sync.dma_start(s_out[:], s_in[:])
            nc.sync.dma_start(a_out[:], a_in[:])
        else:
            nc.gpsimd.collective_compute(
                "AllReduce", mybir.AluOpType.add,
                replica_groups=RG,
                ins=[s_in.opt()], outs=[s_out.opt()],
            )
            nc.gpsimd.collective_compute(
                "AllReduce", mybir.AluOpType.add,
                replica_groups=RG,
                ins=[a_in.opt()], outs=[a_out.opt()],
            )

        # ---- V prep ----
        sprp = cov_ctx.enter_context(tc.tile_pool(name="sprep", bufs=2))
        alod = cov_ctx.enter_context(tc.tile_pool(name="aload", bufs=2))
        with tc.tile_pool(name="opsum", bufs=2, space="PSUM") as opsum:
            a_off = 0
            for w in range(WAY):
                srow = sprp.tile([1, C], F32, tag="sraw")
                nc.sync.dma_start(srow[:], s_out[0:1, w * C:(w + 1) * C])
                sr = sprp.tile([1, C], F32R, tag="sr")
                # s' = s * sqrt(SCALE2 / NTOT); outer(s', s') = SCALE2*s s^T/N
                nc.vector.tensor_scalar_mul(sr[:], srow[:],
                                            float(np.sqrt(SCALE2 / NTOT)))
                a_all = alod.tile([128, sum(VCOLS)], F32, tag="aload")
                nc.sync.dma_start(a_all[:],
                                  a_out[:, a_off:a_off + sum(VCOLS)])
                a_off += sum(VCOLS)
                d_off = 0
                for p in range(CB):
                    keep = (p + 1) * 128
                    o_ps = opsum.tile([128, 640], F32, tag="ops")
                    for lo, hi in VRUNS[p]:
                        nc.tensor.matmul(o_ps[:, lo:hi],
                                         sr[0:1, p * 128:(p + 1) * 128],
                                         sr[0:1, lo:hi],
                                         start=True, stop=True)
                    v = vt[w][p]
                    # V = SCALE2*A - outer   (writes bf16 => rounded)
                    nc.vector.scalar_tensor_tensor(
                        v[:, 0:keep], a_all[:, d_off:d_off + keep], SCALE2,
                        o_ps[:, 0:keep],
                        op0=mybir.AluOpType.mult,
                        op1=mybir.AluOpType.subtract)
                    d_off += keep
                    # mask diagonal block: strict-lower + 0.5*diag
                    nc.vector.tensor_tensor(
                        v[:, p * 128:keep],
                        v[:, p * 128:keep].bitcast(F32), dmask[:],
                        op=mybir.AluOpType.mult)

    # ---------------- phase B: queries ----------------
    qraw = ctx.enter_context(tc.tile_pool(name="qraw", bufs=2))
    qtraw = ctx.enter_context(tc.tile_pool(name="qtraw", bufs=2))
    qnp = ctx.enter_context(tc.tile_pool(name="qn", bufs=2))
    qtnp = ctx.enter_context(tc.tile_pool(name="qtn", bufs=3))
    nrm = ctx.enter_context(tc.tile_pool(name="nrm", bufs=2))
    nrow = ctx.enter_context(tc.tile_pool(name="nrow", bufs=2))
    scratch = ctx.enter_context(tc.tile_pool(name="scr", bufs=3))
    fpool = ctx.enter_context(tc.tile_pool(name="fp", bufs=2))

    with tc.tile_pool(name="ypsum", bufs=3, space="PSUM") as ypsum, \
         tc.tile_pool(name="tpsum", bufs=1, space="PSUM") as tpsum:
        for q in range(QPC):
            # -- load (one 1.25MB DMA), norm --
            raw = qraw.tile([128, CB * D], BF16, tag="qraw")
            nc.sync.dma_start(raw[:], qs_ap[q])
            qn_tiles = []
            inv_cols = []
            for cb in range(CB):
                sq = scratch.tile([128, D], F32, tag="sqscr")
                ssq = nrm.tile([128, 1], F32, tag=f"ssq{cb}")
                nc.scalar.activation(sq[:], raw[:, cb * D:(cb + 1) * D],
                                     mybir.ActivationFunctionType.Square,
                                     accum_out=ssq[:])
                nrm_t = nrm.tile([128, 1], F32, tag=f"nrm{cb}")
                nc.scalar.activation(nrm_t[:], ssq[:],
                                     mybir.ActivationFunctionType.Sqrt)
                nc.vector.tensor_scalar_add(nrm_t[:], nrm_t[:], EPS)
                inv = nrm.tile([128, 1], F32, tag=f"inv{cb}")
                nc.vector.reciprocal(inv[:], nrm_t[:])
                inv_cols.append(inv)
                qn_t = qnp.tile([128, D], F32R, tag=f"qn{cb}")
                nc.scalar.activation(qn_t[:],
                                     raw[:, cb * D:(cb + 1) * D],
                                     mybir.ActivationFunctionType.Copy,
                                     scale=inv[:])
                qn_tiles.append(qn_t)
            # -- inv-norm row [1, C] via PE mini-transposes --
            nu_row = nrow.tile([1, C], F32, tag="nurow")
            for cb in range(CB):
                t_ps = tpsum.tile([1, 128], F32, tag="tps")
                nc.tensor.transpose(t_ps[:], inv_cols[cb][:], idm[:])
                nc.vector.tensor_copy(nu_row[0:1, cb * 128:(cb + 1) * 128],
                                      t_ps[:])
            # broadcast to [128, C] via PE outer product with ones column
            nu_rb = nrow.tile([1, C], BF16, tag="nurb")
            nc.vector.tensor_copy(nu_rb[:], nu_row[:])
            nu_b = nrow.tile([128, C], BF16, tag="nub")
            for lo, hi in ((0, 512), (512, 640)):
                b_ps = tpsum.tile([128, 512], F32, tag="bps")
                nc.tensor.matmul(b_ps[:, 0:hi - lo], ones_row[:],
                                 nu_rb[0:1, lo:hi], start=True, stop=True)
                nc.vector.tensor_copy(nu_b[:, lo:hi], b_ps[:, 0:hi - lo])

            qt_all = qtraw.tile([128, KB * C], BF16, tag="qtraw")
            nc.sync.dma_start(qt_all[:], qt_ap[q])
            f_tiles = [fpool.tile([128, KB], F32, tag=f"f{w}", name=f"f{w}")
                       for w in range(WAY)]

            for kb in range(KB):
                qtn = qtnp.tile([128, C], BF16, tag="qtn")
                nc.vector.tensor_tensor(qtn[:], qt_all[:, kb * C:(kb + 1) * C],
                                        nu_b[:], op=mybir.AluOpType.mult)
                for w in range(WAY):
                    y_ps = ypsum.tile([128, 640], F32, tag="yps")
                    first = {0: True, 1: True}   # per-bank start flags
                    for p in range(CB - 1, -1, -1):
                        lhsT = qn_tiles[p][:, kb * 128:(kb + 1) * 128]
                        for lo, hi in VRUNS[p]:
                            bank = 1 if lo >= 512 else 0
                            is_last = (p == 0)
                            nc.tensor.matmul(
                                y_ps[:, lo:hi], lhsT, vt[w][p][:, lo:hi],
                                start=first[bank],
                                stop=(is_last if bank == 0 else True),
                                skip_group_check=True)
                            first[bank] = False
                    # F[k] = sum_c Y'[k,c] * QnT[k,c]
                    f_t = f_tiles[w]
                    ttr_out = scratch.tile([128, C], F32, tag="ttrscr")
                    nc.vector.scalar_tensor_tensor(
                        ttr_out[:], y_ps[:, 0:C], 1.0, qtn[:],
                        op0=mybir.AluOpType.mult,
                        op1=mybir.AluOpType.mult,
                        accum_out=f_t[:, kb:kb + 1])
                    if kb == KB - 1:
                        x_t = fpool.tile([128, KB], F32, tag="xt")
                        nc.scalar.activation(
                            x_t[:], f_t[:], mybir.ActivationFunctionType.Lrelu,
                            alpha=NEG_SLOPE)
                        cw_scr = scratch.tile([128, KB], F32, tag="cwscr")
                        nc.vector.scalar_tensor_tensor(
                            cw_scr[:], x_t[:], 1.0, cw_sb[:],
                            op0=mybir.AluOpType.mult, op1=mybir.AluOpType.mult,
                            accum_out=r_acc[:, q * WAY + w:q * WAY + w + 1])

    # ---------------- final: scores ----------------
    with tc.tile_pool(name="fin", bufs=1, space="PSUM") as fin, \
         tc.tile_pool(name="osb", bufs=1) as osb:
        sc_ps = fin.tile([1, QPC * WAY], F32, tag="scps")
        nc.tensor.matmul(sc_ps[:], ones32[:], r_acc[:], start=True, stop=True)
        sc_sb = osb.tile([1, QPC * WAY], F32, tag="scsb")
        nc.vector.tensor_scalar_add(sc_sb[:], sc_ps[:], cb_sb[:])
        nc.sync.dma_start(out_ap[:], sc_sb[:])


def _get_nc():
    if "nc" not in _CACHE:
        _CACHE["nc"] = _build()
    return _CACHE["nc"]


def _to_bf16(x):
    import ml_dtypes
    return np.asarray(x, dtype=np.float32).astype(ml_dtypes.bfloat16)


def _host_prep(query, support, conv_w, conv_b):
    q = np.ascontiguousarray(query.reshape(NQ, C, D), dtype=np.float32)
    pad = NCORES * QPC - NQ
    qpad = np.concatenate([q, np.broadcast_to(q[0:1], (pad, C, D))], axis=0)
    # qs: [80][128][CB*D], partition p col-block cb holds channel cb*128+p
    qs_full = np.ascontiguousarray(
        qpad.reshape(NCORES * QPC, CB, 128, D).transpose(0, 2, 1, 3)
        .reshape(NCORES * QPC, 128, CB * D))
    # qt: [80][128][KB*C], partition p col-block kb holds d-index kb*128+p
    qt_full = np.ascontiguousarray(
        qpad.transpose(0, 2, 1).reshape(NCORES * QPC, KB, 128, C)
        .transpose(0, 2, 1, 3).reshape(NCORES * QPC, 128, KB * C))
    # st: per core [128][WAY*NB*C]; col-block (w,nb): sample nb*128+p of way w
    st_full = np.ascontiguousarray(
        support.transpose(0, 2, 1, 3, 4).reshape(WAY, C, NTOT),
        dtype=np.float32)                     # [WAY, C, NTOT]
    st_t = st_full.transpose(0, 2, 1)         # [WAY, NTOT, C]
    cw = np.ascontiguousarray(conv_w.reshape(KB, 128).T, dtype=np.float32)
    cb = np.asarray(conv_b, dtype=np.float32).reshape(1, 1)
    dm = np.tril(np.ones((128, 128), dtype=np.float32), -1) \
        + 0.5 * np.eye(128, dtype=np.float32)
    idm = np.eye(128, dtype=np.float32)
    in_maps = []
    for c in range(NCORES):
        qs = _to_bf16(qs_full[c * QPC:(c + 1) * QPC])
        qt = _to_bf16(qt_full[c * QPC:(c + 1) * QPC])
        stc = st_t[:, c * NCHUNK:(c + 1) * NCHUNK, :]     # [WAY, 640, C]
        st = np.ascontiguousarray(
            stc.reshape(WAY, NB, 128, C).transpose(2, 0, 1, 3)
            .reshape(128, WAY * NB * C), dtype=np.float32)
        in_maps.append({"qs": qs, "qt": qt, "st": st, "cw": cw, "cb": cb,
                        "dm": dm, "idm": idm})
    return in_maps


def kernel(query, support, conv_w, conv_b):
    in_maps = _host_prep(np.asarray(query), np.asarray(support),
                         np.asarray(conv_w), np.asarray(conv_b))
    nc = _get_nc()
    res = run_bass_kernel_spmd(nc, in_maps, core_ids=list(range(NCORES)))
    scores = np.concatenate(
        [res.results[c]["out"].reshape(QPC, WAY) for c in range(NCORES)],
        axis=0)[:NQ]
    return np.ascontiguousarray(scores, dtype=np.float32)


if __name__ == "__main__":
    rng = np.random.default_rng(0)
    import reference
    inputs = reference.setup_inputs()
    exp = np.asarray(reference.reference(**inputs))
    got = kernel(**{k: np.asarray(v) for k, v in inputs.items()})
    rel = np.abs(got - exp).max() / np.abs(exp).max()
    print(f"Relative error: {rel:.3e}")
